# revision 16
# baseline (speedup 1.0000x reference)
"""Trainium2 Bass kernel for CustomMultiHeadAttention.

Problem: B=2, S=2048, D=2048, H=16 heads of Hd=128; y = MHA(q, k, v) with
torch-Linear-style projections (x @ W.T + b) and out projection.

Sharding (8 NeuronCores): data-parallel over batch (2 groups) x tensor-
parallel over heads (4 heads = 512 channels per core). Each core computes
its 4 heads' Q/K/V projections, attention, and a column-sharded partial of
the out projection; the host sums the 4 partials per batch and adds
bo + Wo.T @ bv (the V bias contribution commutes through attention's
convex combination, so it is folded into a host-side constant).

Per-core device program (all matmuls bf16, K=128 contraction):
  - V projected sequence-major first (only needs xv + Wv), then K
    channels-major for all 4 heads, then per 1024-column s-half: Q proj,
    software-pipelined attention, and the partial out-projection.
  - attention inner loop is issue-order pipelined: scores for key-block
    i+2 are issued before P~V of key-block i, so the PE never waits on
    the ACT exp.
  - softmax denominator: DVE sums adjacent exp tiles pairwise (bf16),
    then a ones-column matmul per pair accumulates l in PSUM - half the
    PE stream cost of per-tile ones matmuls.
  - PE idle gaps are filled by interleaving: Q projection blocks for the
    next s-half ride inside attention heads of the current s-half, and
    out-projection blocks of the previous s-half ride inside attention
    heads of the next one.
  - normalization uses reciprocal_approx_fast (18-bit) + gpsimd
    partition broadcast, multiplied into O~ straight from PSUM.
"""

import math

import numpy as np
import ml_dtypes

B = 2
S = 2048
D = 2048
HD = 128          # head dim
TP = 4            # head-group (tensor-parallel) factor
CL = D // TP      # 512 local channels = 4 heads per core
NCORES = 8

_NC = None


def _build_nc(s=S, d=D, cl=CL):
    """Build the per-core Bass program (SPMD: same program, 8 cores)."""
    from contextlib import ExitStack

    import concourse.bass as bass
    import concourse.mybir as mybir
    import concourse.tile as tile
    from concourse import bacc, bass_isa

    f32 = mybir.dt.float32
    bf16 = mybir.dt.bfloat16
    Exp = mybir.ActivationFunctionType.Exp

    SBW = 512                 # single matmul max free dim / PSUM bank width
    W2 = 2 * SBW              # paired two-bank tile width
    NSB = s // SBW            # 4 s-blocks of 512
    NSP = s // W2             # 2 s-halves of 1024
    NTB = s // 128            # 16 key/value seq blocks
    KC = d // 128             # 16 contraction chunks over model dim
    NH = cl // HD             # 4 local heads
    SCALE = 1.0 / math.sqrt(HD)

    nc = bacc.Bacc("TRN2", target_bir_lowering=False, debug=False)

    xqT = nc.dram_tensor("xqT", [d, s], bf16, kind="ExternalInput").ap()
    xkT = nc.dram_tensor("xkT", [d, s], bf16, kind="ExternalInput").ap()
    xvT = nc.dram_tensor("xvT", [d, s], bf16, kind="ExternalInput").ap()
    wqT = nc.dram_tensor("wqT", [d, cl], bf16, kind="ExternalInput").ap()
    wkT = nc.dram_tensor("wkT", [d, cl], bf16, kind="ExternalInput").ap()
    wvT = nc.dram_tensor("wvT", [d, cl], bf16, kind="ExternalInput").ap()
    woT = nc.dram_tensor("woT", [cl, d], bf16, kind="ExternalInput").ap()
    bq = nc.dram_tensor("bq", [cl], f32, kind="ExternalInput").ap()
    bk = nc.dram_tensor("bk", [cl], f32, kind="ExternalInput").ap()
    zT = nc.dram_tensor("zT", [d, s], bf16, kind="ExternalOutput").ap()

    with tile.TileContext(nc) as tc, ExitStack() as ctx:
        const = ctx.enter_context(tc.tile_pool(name="const", bufs=1))
        wp = ctx.enter_context(tc.tile_pool(name="weights", bufs=4))
        kvp = ctx.enter_context(tc.tile_pool(name="kv", bufs=1))
        panels = ctx.enter_context(tc.tile_pool(name="panels", bufs=6))
        qtp = ctx.enter_context(tc.tile_pool(name="qt", bufs=1))
        otp = ctx.enter_context(tc.tile_pool(name="ot", bufs=2))
        p2p = ctx.enter_context(tc.tile_pool(name="ptiles", bufs=7))
        pap = ctx.enter_context(tc.tile_pool(name="padd", bufs=2))
        smalls = ctx.enter_context(tc.tile_pool(name="small", bufs=1))
        zpool = ctx.enter_context(tc.tile_pool(name="zout", bufs=3))
        # PSUM: rotating pair-bank pool (4) + PV accum (2) + l rows (1)
        ps_big = ctx.enter_context(tc.tile_pool(name="ps_big", bufs=2,
                                                space="PSUM"))
        ps_ops = ctx.enter_context(tc.tile_pool(name="ps_ops", bufs=1,
                                                space="PSUM"))
        ps_l = ctx.enter_context(tc.tile_pool(name="ps_l", bufs=1,
                                              space="PSUM"))
        ps_q = ctx.enter_context(tc.tile_pool(name="ps_q", bufs=1,
                                              space="PSUM"))

        ones_col = const.tile([128, 1], bf16, tag="ones_col")
        nc.vector.memset(ones_col, 1.0)
        warm = const.tile([128, SBW], bf16, tag="warm")
        nc.vector.memset(warm, 0.0)
        ps_warm = ps_q.tile([128, SBW], f32, tag="q", name="ps_warm")
        for _ in range(58):
            nc.tensor.matmul(ps_warm[0:1, :], lhsT=ones_col, rhs=warm,
                             start=True, stop=True, skip_group_check=True)

        def load_bias(b_dram, nm):
            b_dma = const.tile([128, NH], f32, tag=f"{nm}d", name=f"{nm}d")
            nc.scalar.dma_start(b_dma, b_dram.rearrange("(m p) -> p m", p=128))
            b_sb = const.tile([128, NH], f32, tag=nm, name=nm)
            nc.vector.tensor_copy(b_sb, b_dma)
            return b_sb

        bq_sb = load_bias(bq, "bq")
        bk_sb = load_bias(bk, "bk")

        KC2 = KC // 2

        # Weights and panels are split into k-halves (separate tiles) so
        # consumers can start as soon as the first half lands.
        def wpair(w_dram, nm):
            r = w_dram.rearrange("(k p) m -> p k m", p=128)
            pair = []
            for i in range(2):
                w = wp.tile([128, KC2, cl], bf16, tag="w", name=f"{nm}{i}")
                nc.sync.dma_start(w, r[:, i * KC2:(i + 1) * KC2])
                pair.append(w)
            return pair

        def wap(pair, k, m):
            return pair[k // KC2][:, k % KC2, m * 128:(m + 1) * 128]

        # V path first: its weight + first panel gate the first matmul.
        wv_sb = wpair(wvT, "wv")

        kt = [kvp.tile([128, s], bf16, tag=f"kt{h}", name=f"kt{h}")
              for h in range(NH)]
        vt = [kvp.tile([128, cl], bf16, tag=f"vt{t}", name=f"vt{t}")
              for t in range(NTB)]

        def xpanel(x_dram, n, nm):
            src_ = x_dram[:, n * SBW:(n + 1) * SBW].rearrange(
                "(k p) t -> p k t", p=128)
            xps = []
            for i in range(2):
                xp = panels.tile([128, KC2, SBW], bf16, tag="xpanel",
                                 name=f"{nm}_{i}")
                nc.sync.dma_start(xp, src_[:, i * KC2:(i + 1) * KC2])
                xps.append(xp)
            return xps

        def xap(pair, k):
            return pair[k // KC2][:, k % KC2, :]

        # ---- V projection: vt[t][tt, e] = sum_d x[t*128+tt, d] Wv[e, d]
        for n in range(NSB):
            xp = xpanel(xvT, n, f"xpv{n}")
            for pair in range(2):
                ps = ps_big.tile([128, W2], f32, tag="mm", name="ps_v")
                for half in range(2):
                    tsub = pair * 2 + half
                    for k in range(KC):
                        nc.tensor.matmul(
                            ps[:, half * SBW:(half + 1) * SBW],
                            lhsT=xap(xp, k)[:, tsub * 128:(tsub + 1) * 128],
                            rhs=wv_sb[k // KC2][:, k % KC2, :],
                            start=(k == 0), stop=(k == KC - 1))
                for half in range(2):
                    t = n * 4 + pair * 2 + half
                    nc.vector.tensor_copy(
                        vt[t], ps[:, half * SBW:(half + 1) * SBW])

        # ---- K projection (channels-major, all heads, 512-col blocks) --
        wk_sb = wpair(wkT, "wk")

        def proj_mms(w_sb, xp, m, pool_tag="mm"):
            pool = ps_q if pool_tag == "q" else ps_big
            ps = pool.tile([128, SBW], f32, tag=pool_tag, name="ps_proj")
            for k in range(KC):
                nc.tensor.matmul(
                    ps, lhsT=wap(w_sb, k, m),
                    rhs=xap(xp, k), start=(k == 0), stop=(k == KC - 1))
            return ps

        def proj_block(w_sb, b_sb, xp, m, out_ap):
            # out_ap [128, 512] = W_m x(+bias) for one 512-col s-block
            ps = proj_mms(w_sb, xp, m)
            nc.vector.tensor_scalar_add(out_ap, ps, b_sb[:, m:m + 1])

        def filler_qblock(m, qt_next, xq_next):
            # filler Q-proj: psum from the spare bank + ring, bias on ACT
            psA = proj_mms(wq_sb, xq_next[0], m, pool_tag="q")
            psB = proj_mms(wq_sb, xq_next[1], m)
            for b, ps in ((0, psA), (1, psB)):
                nc.scalar.activation(
                    qt_next[m][:, b * SBW:(b + 1) * SBW], ps,
                    mybir.ActivationFunctionType.Identity,
                    bias=bq_sb[:, m:m + 1])

        for b in range(NSB):
            xp = xpanel(xkT, b, f"xpk{b}")
            for m in range(NH):
                proj_block(wk_sb, bk_sb, xp, m,
                           kt[m][:, b * SBW:(b + 1) * SBW])

        # Q panels for s-half 0, then wq/wo (ring-slot gated behind wv/wk)
        xq_panels = [xpanel(xqT, 0, "xpq0"), xpanel(xqT, 1, "xpq1")]
        wq_sb = wpair(wqT, "wq")
        rwo = woT.rearrange("(k p) m -> p k m", p=128)
        wo_sb = []
        for i in range(2):
            w = wp.tile([128, NH // 2, d], bf16, tag="w", name=f"wo{i}")
            nc.sync.dma_start(w, rwo[:, i * (NH // 2):(i + 1) * (NH // 2)])
            wo_sb.append(w)

        qt_cur = [qtp.tile([128, W2], bf16, tag=f"qt{h}", name=f"qt{h}_0")
                  for h in range(NH)]
        for b in range(2):
            for m in range(NH):
                proj_block(wq_sb, bq_sb, xq_panels[b], m,
                           qt_cur[m][:, b * SBW:(b + 1) * SBW])

        ot_prev = None
        qt_next = None
        xq_next = None

        def outproj_dd(dd, sp, ot_tiles, cast_dve=False):
            # z[dd-block, s-half sp] accumulated over all 4 local heads
            ps = ps_big.tile([128, W2], f32, tag="mm", name=f"ps_z{dd}")
            for half in range(2):
                for eb in range(NH):
                    wo_ap = wo_sb[eb // 2][:, eb % 2,
                                           dd * 128:(dd + 1) * 128]
                    nc.tensor.matmul(
                        ps[:, half * SBW:(half + 1) * SBW],
                        lhsT=wo_ap,
                        rhs=ot_tiles[eb][:, half * SBW:(half + 1) * SBW],
                        start=(eb == 0), stop=(eb == NH - 1),
                        skip_group_check=True)
            zt = zpool.tile([128, W2], bf16, tag="z", name=f"z{dd}")
            if cast_dve:
                nc.vector.tensor_copy(zt, ps)
            else:
                nc.scalar.activation(
                    zt, ps, mybir.ActivationFunctionType.Copy)
            nc.sync.dma_start(
                zT[dd * 128:(dd + 1) * 128, sp * W2:(sp + 1) * W2], zt)

        # ---- attention per (s-half, head) with PE filler blocks --------
        for sp in range(NSP):
            ot = [otp.tile([128, W2], bf16, tag=f"ot{h}", name=f"ot{h}_{sp}")
                  for h in range(NH)]
            if sp == 0:
                # prefetch Q panels + allocate qt for s-half 1; its proj
                # blocks are interleaved into this half's attention heads
                xq_next = [xpanel(xqT, 2, "xpq2"), xpanel(xqT, 3, "xpq3")]
                qt_next = [qtp.tile([128, W2], bf16, tag=f"qt{h}",
                                    name=f"qt{h}_1") for h in range(NH)]

            for h in range(NH):
                # filler PE blocks (no ACT dependence) at head start
                if sp == 0:
                    if h > 0:
                        filler_qblock(h - 1, qt_next, xq_next)
                else:
                    if h == 0:
                        filler_qblock(3, qt_next, xq_next)
                    outproj_pre = [4 * h + 0, 4 * h + 1]
                    outproj_later = [4 * h + 2, 4 * h + 3]

                ops = ps_ops.tile([128, W2], f32, tag="ops", name="ps_pv")
                lps = ps_l.tile([33, SBW], f32, tag="l", name="ps_l")
                p2 = [None] * NTB
                pd = [None] * (NTB // 2)
                qd = [None] * (NTB // 4)

                def sc_exp(i, h=h, p2=p2):
                    ps = ps_big.tile([128, W2], f32, tag="mm",
                                     name=f"ps_sc{i}")
                    kb = kt[h][:, i * 128:(i + 1) * 128]
                    qth = qt_cur[h]
                    for half in range(2):
                        nc.tensor.matmul(
                            ps[:, half * SBW:(half + 1) * SBW],
                            lhsT=kb,
                            rhs=qth[:, half * SBW:(half + 1) * SBW],
                            start=True, stop=True)
                    p2[i] = p2p.tile([128, W2], bf16, tag="p",
                                     name=f"p{h}_{i}")
                    nc.scalar.activation(p2[i], ps, Exp, scale=SCALE)

                def pv(i, h=h, ops=ops, p2=p2, pd=pd):
                    vb = vt[i][:, h * 128:(h + 1) * 128]
                    for half in range(2):
                        nc.tensor.matmul(
                            ops[:, half * SBW:(half + 1) * SBW],
                            lhsT=vb,
                            rhs=p2[i][:, half * SBW:(half + 1) * SBW],
                            start=(i == 0), stop=(i == NTB - 1),
                            skip_group_check=True)
                    if i % 2 == 1:
                        j = i // 2
                        pd[j] = pap.tile([128, W2], bf16, tag="pd",
                                         name=f"pd{j}")
                        nc.vector.tensor_add(pd[j], p2[i - 1], p2[i])
                    if i % 4 == 3:
                        m2 = i // 4
                        qd[m2] = pap.tile([128, W2], bf16, tag="qd",
                                          name=f"qd{m2}")
                        nc.vector.tensor_add(qd[m2], pd[2 * m2],
                                             pd[2 * m2 + 1])

                def lsum(m2, lps=lps, qd=qd):
                    for half in range(2):
                        nc.tensor.matmul(
                            lps[32 * half:32 * half + 1, :],
                            lhsT=ones_col,
                            rhs=qd[m2][:, half * SBW:(half + 1) * SBW],
                            start=(m2 == 0), stop=(m2 == NTB // 4 - 1),
                            skip_group_check=True)

                sc_exp(0)
                if sp == 1:
                    for dd in outproj_pre:
                        outproj_dd(dd, 0, ot_prev)
                sc_exp(1)
                for i in range(NTB):
                    if i + 2 < NTB:
                        sc_exp(i + 2)
                    pv(i)
                    if i in (5, 9, 13):
                        lsum((i - 5) // 4)
                    if sp == 1 and i in (3, 7):
                        outproj_dd(outproj_later[(i - 3) // 4], 0, ot_prev,
                                   cast_dve=True)
                lsum(NTB // 4 - 1)

                # drain O~ off PSUM fast (frees accumulator for next head;
                # the very last head reads PSUM directly instead)
                last_head = (sp == NSP - 1 and h == NH - 1)
                if not last_head:
                    o_raw = smalls.tile([128, W2], f32, tag="o_raw",
                                        name=f"o_raw{h}")
                    nc.vector.tensor_copy(o_raw, ops)
                else:
                    o_raw = ops
                # normalize: 1/l broadcast, multiply
                l_sb = smalls.tile([1, W2], f32, tag="l_sb", name="l_sb")
                r_sb = smalls.tile([1, W2], f32, tag="r_sb", name="r_sb")
                rb = smalls.tile([128, W2], f32, tag="rb", name="rb")
                for half in range(2):
                    hs = slice(half * SBW, (half + 1) * SBW)
                    nc.vector.tensor_copy(l_sb[:, hs], lps[32 * half:32 * half + 1, :])
                    nc.vector.reciprocal_approx_fast(r_sb[:, hs], l_sb[:, hs])
                    nc.gpsimd.partition_broadcast(rb[:, hs], r_sb[:, hs])
                    nc.vector.tensor_mul(ot[h][:, hs], o_raw[:, hs], rb[:, hs])

            if sp == 0:
                ot_prev = ot
                qt_cur = qt_next
            else:
                # keep the PE p-state hot across the last normalize chain
                ps_warm2 = ps_q.tile([128, SBW], f32, tag="q",
                                     name="ps_warm2")
                for _ in range(10):
                    nc.tensor.matmul(ps_warm2[0:1, :], lhsT=ones_col,
                                     rhs=warm, start=True, stop=True,
                                     skip_group_check=True)
                # tail: out-projection for the last s-half. First two
                # blocks defer their eb=3 matmuls so the PE has work while
                # the last head's normalization finishes; final blocks
                # drain at half width so the cast/DMA tail is shorter.
                first_ps = []
                for dd in range(2):
                    ps = ps_big.tile([128, W2], f32, tag="mm",
                                     name=f"ps_z{dd}")
                    first_ps.append(ps)
                    for half in range(2):
                        for eb in range(NH - 1):
                            nc.tensor.matmul(
                                ps[:, half * SBW:(half + 1) * SBW],
                                lhsT=wo_sb[eb // 2][:, eb % 2,
                                                    dd * 128:(dd + 1) * 128],
                                rhs=ot[eb][:, half * SBW:(half + 1) * SBW],
                                start=(eb == 0), stop=False,
                                skip_group_check=True)
                for dd in range(2):
                    ps = first_ps[dd]
                    for half in range(2):
                        nc.tensor.matmul(
                            ps[:, half * SBW:(half + 1) * SBW],
                            lhsT=wo_sb[1][:, 1, dd * 128:(dd + 1) * 128],
                            rhs=ot[3][:, half * SBW:(half + 1) * SBW],
                            start=False, stop=True, skip_group_check=True)
                    zt = zpool.tile([128, W2], bf16, tag="z", name=f"z{dd}")
                    eng = nc.scalar if dd % 2 == 0 else nc.vector
                    if dd % 2 == 0:
                        nc.scalar.activation(
                            zt, ps, mybir.ActivationFunctionType.Copy)
                    else:
                        nc.vector.tensor_copy(zt, ps)
                    nc.sync.dma_start(
                        zT[dd * 128:(dd + 1) * 128, W2:2 * W2], zt)
                for dd in range(2, KC - 4):
                    outproj_dd(dd, 1, ot, cast_dve=(dd % 2 == 1))
                for dd in range(KC - 4, KC):
                    for half in range(2):
                        hs = slice(half * SBW, (half + 1) * SBW)
                        ps = ps_big.tile([128, SBW], f32, tag="mm",
                                         name=f"ps_zf{dd}_{half}")
                        for eb in range(NH):
                            nc.tensor.matmul(
                                ps,
                                lhsT=wo_sb[eb // 2][:, eb % 2,
                                                    dd * 128:(dd + 1) * 128],
                                rhs=ot[eb][:, hs],
                                start=(eb == 0), stop=(eb == NH - 1),
                                skip_group_check=True)
                        zt = zpool.tile([128, SBW], bf16, tag="z",
                                        name=f"zf{dd}_{half}")
                        if half == 0:
                            nc.scalar.activation(
                                zt, ps, mybir.ActivationFunctionType.Copy)
                        else:
                            nc.vector.tensor_copy(zt, ps)
                        nc.sync.dma_start(
                            zT[dd * 128:(dd + 1) * 128,
                               W2 + half * SBW:W2 + (half + 1) * SBW], zt)

    nc.compile()
    return nc


def _bf16(a):
    return np.ascontiguousarray(a).astype(ml_dtypes.bfloat16)


def _in_maps(inputs):
    q = np.asarray(inputs["query"], dtype=np.float32)
    k = np.asarray(inputs["key_in"], dtype=np.float32)
    v = np.asarray(inputs["value"], dtype=np.float32)
    Wq = np.asarray(inputs["Wq"], dtype=np.float32)
    Wk = np.asarray(inputs["Wk"], dtype=np.float32)
    Wv = np.asarray(inputs["Wv"], dtype=np.float32)
    Wo = np.asarray(inputs["Wo"], dtype=np.float32)
    bq = np.asarray(inputs["bq"], dtype=np.float32)
    bk = np.asarray(inputs["bk"], dtype=np.float32)

    xT = [[_bf16(x[b].T) for b in range(B)] for x in (q, k, v)]
    maps = []
    for c in range(NCORES):
        b, g = divmod(c, TP)
        sl = slice(g * CL, (g + 1) * CL)
        maps.append({
            "xqT": xT[0][b], "xkT": xT[1][b], "xvT": xT[2][b],
            "wqT": _bf16(Wq[sl, :].T), "wkT": _bf16(Wk[sl, :].T),
            "wvT": _bf16(Wv[sl, :].T), "woT": _bf16(Wo[:, sl].T),
            "bq": np.ascontiguousarray(bq[sl]),
            "bk": np.ascontiguousarray(bk[sl]),
        })
    return maps


TRACE = False
TMPDIR = None
LAST_RESULT = None


def kernel(**inputs):
    global _NC, LAST_RESULT
    from concourse.bass_utils import run_bass_kernel_spmd

    if _NC is None:
        _NC = _build_nc()
    maps = _in_maps(inputs)
    res = run_bass_kernel_spmd(_NC, maps, core_ids=list(range(NCORES)),
                               trace=TRACE, tmpdir=TMPDIR)
    LAST_RESULT = res

    Wo = np.asarray(inputs["Wo"], dtype=np.float32)
    bv = np.asarray(inputs["bv"], dtype=np.float32)
    bo = np.asarray(inputs["bo"], dtype=np.float32)
    out = np.zeros((B, S, D), dtype=np.float32)
    for c in range(NCORES):
        b, _ = divmod(c, TP)
        out[b] += res.results[c]["zT"].astype(np.float32).T
    out += (Wo @ bv + bo)[None, None, :]
    return out


if __name__ == "__main__":
    _build_nc()
    print("build OK")
